# revision 11
# baseline (speedup 1.0000x reference)
"""Trainium2 Bass kernel for nn_CrossCorrelLoss.

Math: for input X of shape (B=32, T=1024, D=321) the reference computes
  mu, sd over all (B,T) per feature; Xs = (X-mu)/sd;
  ccf = mean_b [Xs_b^T Xs_b / T]  (lower-triangle entries);
  loss = sum |ccf_fake - ccf_real| / 10.
mean_b of the per-batch Grams equals the flat Gram over all N=B*T rows, so
  loss = sum_tril |Gf - Gr| / 10,  G = Xs^T Xs / (B*T).

Host marshalling: standardize (mu/sd in float64 — O(B*T*D) elementwise
preprocessing, same order as the dtype cast) and quantize to fp8 e4m3
(ml_dtypes.float8_e4m3 == TRN FP8_EXP4: max +-240, our data is ~N(0,1),
max |x| ~ 5.4, so no clipping concerns). Validated end-to-end: the fp8
quantization noise lands at ~4e-4 relative on the scalar loss vs the 2e-2
gate. This cuts device HBM traffic 4x vs streaming fp32 (2.63 MB/core).

Device (data-parallel over batch, 4 batches = 4096 rows per core):
per input, the upper-triangle blocks of the raw Gram S = Xs^T Xs over the
local rows, via fp8 DoubleRow matmuls: each instruction contracts a
256-row chunk (2 k-tiles of 128 packed per partition) at ~2x bf16
throughput. Triangle blocking (row x col):
    b0: 0:128   x 0:321    (N=321)  DoubleRow
    b1: 128:256 x 128:321  (N=193)  DoubleRow
    b2: 256:321 x 256:321  (N=65)   normal mode x2 k-tiles (FD<128 =>
                                     DoubleRow loses: it disables FWL)
PSUM accumulates fp32 over the 16 chunks; results pack into one
(128, 579) bf16 tile per input and DMA back.

DoubleRow ISA constraints (s3_lw/s3d3_mm dual_fp8_restrictions): the AP
pair dim must have n_elem==2 and step % 16 == 0. SBUF layout is
[p, i, c, d] with i the k-tile pair dim: step_i = 16*321 = 5136 bytes.
Host pre-permutes rows so partition p, pair i, chunk c holds data row
c*256 + i*128 + p (row order is irrelevant for a Gram).

Host: sum the 8 per-core partial Grams in float64 (the all-reduce over
batch), mirror the triangle, G/(B*T), tril abs-sum / 10.
"""

import numpy as np
import ml_dtypes

import concourse.bacc as bacc
import concourse.bass as bass
import concourse.mybir as mybir
import concourse.tile as tile
from concourse import bass_utils

N_CORES = 8
B, T, D = 32, 1024, 321
ROWS_PER_CORE = (B // N_CORES) * T  # 4096
CH = 256  # rows per DoubleRow chunk (2 k-tiles of 128)
NCH = ROWS_PER_CORE // CH  # 16
NTOT = B * T  # 32768

FP8 = mybir.dt.float8e4
NP_FP8 = ml_dtypes.float8_e4m3
ACC_DT = mybir.dt.float32
# Partial Grams travel back as bf16: the host sums them in float64; the
# ~2^-9 per-entry rounding averages out to ~1e-5 on the final scalar.
ST_DT = mybir.dt.bfloat16

# Upper-triangle row blocks of the Gram: (row_lo, row_hi, col_lo, col_hi)
TRI_BLOCKS = [(0, 128, 0, D), (128, 256, 128, D), (256, D, 256, D)]
# staging-column offset of each block in the packed (128, 579) output
TRI_OFF = [0, D, D + (D - 128)]
OUT_W = sum(hi - lo for _, _, lo, hi in TRI_BLOCKS)  # 579

# Input DMA split, in chunk-groups (sum = 16). Graduated: small first
# transfers land early so the PE starts chasing sooner; later ones are
# big for descriptor efficiency.
DMA_GROUPS = [(0, 1), (1, 2), (2, 4), (4, 8), (8, 12), (12, 16)]

import os
# all-DoubleRow (b2 as one DR matmul) vs b2 in normal mode x2 k-passes
B2_DR = os.environ.get("K_B2_DR", "1") == "1"
# per-chunk block emission order when all-DR
B2_DR_ORDER = tuple(int(c) for c in os.environ.get("K_ORDER", "012"))
# PE clock warmup matmuls emitted once before the main stream: the HAM
# ramps the PE 0.65 -> 1.2 -> 2.4 GHz over ~3us of continuous activity,
# and the PE would otherwise idle during the initial DMA fill.
WARMUP_MMS = int(os.environ.get("K_WARMUP", "5"))

_NC_CACHE = {}


def _emit_input(nc, tv_src, xpool, base, rnd):
    """DMA the [128, 2, 16, 321] fp8 block for one input; returns the tile."""
    t = xpool.tile(
        [128, 2, NCH, D], FP8, name=f"{base}_t_r{rnd}", tag=f"{base}_t", bufs=2
    )
    for c0, c1 in DMA_GROUPS:
        nc.sync.dma_start(out=t[:, :, c0:c1, :], in_=tv_src[:, :, c0:c1, :])
    return t


def _emit_gram(nc, t, ppool, spool, g_out, base, rnd, cast_engines):
    """Triangle-blocked Gram of one input tile + packed bf16 store."""
    psums = []
    for bi, (rlo, rhi, clo, chi) in enumerate(TRI_BLOCKS):
        psums.append(
            ppool.tile(
                [rhi - rlo, chi - clo],
                ACC_DT,
                name=f"{base}_ps{bi}_r{rnd}",
                tag=f"{base}_ps{bi}",
            )
        )

    def dr_mm(bi, c):
        rlo, rhi, clo, chi = TRI_BLOCKS[bi]
        nc.tensor.matmul(
            psums[bi][:, :],
            t[:, :, c, rlo:rhi],
            t[:, :, c, clo:chi],
            start=c == 0,
            stop=c == NCH - 1,
            perf_mode=mybir.MatmulPerfMode.DoubleRow,
        )

    # Chunk pairs, with both b2 matmuls emitted back-to-back at the end:
    # the only LDWEIGHTS the PE's one background weight buffer cannot hide
    # is the one issued right after the short b2 stream (73 cycles) — this
    # order halves how often a long DoubleRow weight load lands there.
    for c in range(0, NCH, 2):
        for bi in (0, 1):
            dr_mm(bi, c)
        for bi in (0, 1):
            dr_mm(bi, c + 1)
        dr_mm(2, c)
        dr_mm(2, c + 1)

    st = spool.tile([128, OUT_W], ST_DT, name=f"{base}_st_r{rnd}", tag=f"{base}_st", bufs=2)
    for bi, (rlo, rhi, clo, chi) in enumerate(TRI_BLOCKS):
        cast_engines[bi % 2](
            st[0 : rhi - rlo, TRI_OFF[bi] : TRI_OFF[bi] + chi - clo],
            psums[bi][:, :],
        )
    # SWDGE (gpsimd) keeps the store off the SP HWDGE ring that feeds the
    # input loads — a HWDGE sequencer stalls on the store's sem wait
    # (casts <- all matmuls), which would block the next loads behind it.
    nc.gpsimd.dma_start(out=g_out[:, :], in_=st[:, :])


def _build_program(n_rounds: int = 1, dma_once: bool = False, dma_only: bool = False):
    # n_rounds > 1 repeats the whole pipeline inside one NEFF — used only
    # by bench.py to measure steady-state HW time via the repetition
    # slope, which cancels the per-call axon RPC overhead.
    nc = bacc.Bacc(trn_type="TRN2", target_bir_lowering=False, debug=False)

    ins = {}
    outs = {}
    for key in ("xf", "xr"):
        ins[key] = (
            nc.dram_tensor(key, [128, 2 * NCH * D], FP8, kind="ExternalInput")
            .ap()
            .rearrange("p (i c d) -> p i c d", i=2, d=D)
        )
        outs[key] = nc.dram_tensor(
            "g" + key[1], [128, OUT_W], ST_DT, kind="ExternalOutput"
        ).ap()

    with tile.TileContext(nc) as tc:
        with (
            tc.tile_pool(name="x", bufs=1) as xpool,
            tc.tile_pool(name="ps", bufs=1, space=bass.MemorySpace.PSUM) as ppool,
            tc.tile_pool(name="st", bufs=1) as spool,
        ):
            cast_engines = [nc.vector.tensor_copy, nc.scalar.copy]

            if WARMUP_MMS and not dma_only:
                wt = xpool.tile([128, 2, 512], FP8, name="warm_t")
                nc.gpsimd.memset(wt[:, :, :], 0)
                wp = ppool.tile([128, 512], ACC_DT, name="warm_ps")
                for wi in range(WARMUP_MMS):
                    nc.tensor.matmul(
                        wp[:, :],
                        wt[:, :, 0:128],
                        wt[:, :, :],
                        start=True,
                        stop=True,
                        perf_mode=mybir.MatmulPerfMode.DoubleRow,
                    )

            static_tiles = {}
            if dma_once:
                for base in ("xf", "xr"):
                    static_tiles[base] = _emit_input(nc, ins[base], xpool, base, 0)

            for rnd in range(n_rounds):
                for base in ("xf", "xr"):
                    if dma_once:
                        t = static_tiles[base]
                    else:
                        t = _emit_input(nc, ins[base], xpool, base, rnd)
                    if dma_only:
                        continue
                    _emit_gram(
                        nc, t, ppool, spool, outs[base], base, rnd, cast_engines
                    )

    nc.compile()
    return nc


def _marshal(x: np.ndarray) -> list[np.ndarray]:
    """Standardize, quantize to fp8, shard by batch, permute to [p,i,c,d]."""
    x = np.asarray(x, dtype=np.float32).reshape(NTOT, D)
    mu = x.mean(axis=0, dtype=np.float64)
    var = x.var(axis=0, dtype=np.float64) * (NTOT / (NTOT - 1.0))
    scale = (1.0 / np.sqrt(var)).astype(np.float32)
    xs = (x - mu.astype(np.float32)) * scale
    q = xs.astype(NP_FP8)
    # rows for core k: [k*4096, (k+1)*4096); row c*256 + i*128 + p -> [p,i,c,d]
    q5 = q.reshape(N_CORES, NCH, 2, 128, D).transpose(0, 3, 2, 1, 4)
    return [q5[k].reshape(128, 2 * NCH * D).copy() for k in range(N_CORES)]


def _assemble(packed: np.ndarray) -> np.ndarray:
    """(128, 579) packed triangle blocks -> full symmetric (321, 321)."""
    s = np.zeros((D, D), dtype=np.float64)
    for bi, (rlo, rhi, clo, chi) in enumerate(TRI_BLOCKS):
        s[rlo:rhi, clo:chi] = packed[0 : rhi - rlo, TRI_OFF[bi] : TRI_OFF[bi] + chi - clo]
    return s


def kernel(x_fake: np.ndarray, x_real: np.ndarray, _trace=False):
    if "nc" not in _NC_CACHE:
        _NC_CACHE["nc"] = _build_program()
    nc = _NC_CACHE["nc"]

    fs = _marshal(x_fake)
    rs = _marshal(x_real)
    in_maps = [{"xf": fs[c], "xr": rs[c]} for c in range(N_CORES)]

    res = bass_utils.run_bass_kernel_spmd(
        nc, in_maps, core_ids=list(range(N_CORES)), trace=_trace
    )

    sf = np.zeros((D, D), dtype=np.float64)
    sr = np.zeros((D, D), dtype=np.float64)
    for c in range(N_CORES):
        sf += _assemble(res.results[c]["gf"].astype(np.float64))
        sr += _assemble(res.results[c]["gr"].astype(np.float64))

    # the device computes the upper-triangle blocks; G is symmetric so the
    # reference's tril sum equals the triu sum
    i0, i1 = np.triu_indices(D)
    diff = (sf - sr) / float(NTOT)
    loss = np.float32(np.abs(diff[i0, i1]).sum() / 10.0)
    if _trace:
        return loss, res
    return loss
